# revision 2
# baseline (speedup 1.0000x reference)
"""BloomAttention (B=1, S=2048, H=4096, NH=32) on 8 Trainium2 cores — v3.

Head-parallel TP (4 heads/core), all matmul operands bf16 (fp32 PSUM accum).
 - QKV: PSUM-resident accumulation over the full 4096 contraction per
   256-wide seq chunk (no SBUF/DVE accumulate passes).
 - scores: QK matmul + a 3-row aux matmul (ones3 x [alibi_hi;mid;lo] — exact
   3-way bf16 split of alibi) + a tri^T x identity matmul for the causal
   diagonal block. Everything lands in PSUM; no DVE score pass at all.
 - softmax: exp straight from PSUM on the scalar engine with a
   host-precomputed per-query upper bound as bias (replaces the row max;
   the shift cancels in normalization), accum_out gives row sums.
 - probs normalized by 1/sum (DVE), transposed to key-major via DMA X-bar
   transposes (USE_DMA_T) or PE transposes + copies (fallback).
 - dense: row-parallel partials written bf16; host sums cores+bias+residual.
"""
import math
import numpy as np
from contextlib import ExitStack

import ml_dtypes

import concourse.bacc as bacc
import concourse.bass as bass
import concourse.mybir as mybir
import concourse.tile as tile
from concourse.bass_utils import run_bass_kernel_spmd

B, S, H, NH = 1, 2048, 4096, 32
HD = H // NH            # 128
NCORES = 8
HPC = NH // NCORES      # 4 heads per core
DPC = HPC * HD          # 512
INV_NORM = 1.0 / math.sqrt(HD)
NEG = -1.0e30
CPAD = 15.0             # slack above max alibi in b_q
P = 128
QB = S // P             # 16 query blocks
NCH = 8                 # seq chunks in phase 1
SCW = S // NCH          # 256 seq chunk width
KT = H // P             # 32 contraction tiles
F32 = mybir.dt.float32
BF16 = mybir.dt.bfloat16
ADD = mybir.AluOpType.add

USE_DMA_T = False        # DMA X-bar transposes vs PE transposes

_CACHE = {}


def _build(key):
    kNq, kLoT = key
    nc = bacc.Bacc("TRN2", target_bir_lowering=False, debug=False,
                   num_devices=NCORES)

    hpk = nc.dram_tensor("hpk", [NCH, P, KT * SCW], BF16, kind="ExternalInput")
    wqk = nc.dram_tensor("wqk", [2 * HPC, P, KT * P], BF16, kind="ExternalInput")
    wv = nc.dram_tensor("wv", [P, KT * DPC], BF16, kind="ExternalInput")
    bqk_t = nc.dram_tensor("bqk", [P, 2 * HPC], F32, kind="ExternalInput")
    bv_t = nc.dram_tensor("bv", [1, DPC], F32, kind="ExternalInput")
    alsp_t = nc.dram_tensor("alsp", [HPC, 3, S], BF16, kind="ExternalInput")
    trit_t = nc.dram_tensor("trit", [QB, P, P], BF16, kind="ExternalInput")
    bexp_t = nc.dram_tensor("bexp", [P, HPC * QB], F32, kind="ExternalInput")
    ident_t = nc.dram_tensor("ident", [P, P], BF16, kind="ExternalInput")
    wdp_t = nc.dram_tensor("wdp", [P, HPC * H], BF16, kind="ExternalInput")
    out_t = nc.dram_tensor("out_part", [S, H], BF16, kind="ExternalOutput")

    Ident = mybir.ActivationFunctionType.Identity
    Exp = mybir.ActivationFunctionType.Exp

    with tile.TileContext(nc) as tc, ExitStack() as top:
        persist = top.enter_context(tc.tile_pool(name="persist", bufs=1))
        qk_sb = [persist.tile([P, S], BF16, tag=f"qk_{f}", name=f"qk_{f}")
                 for f in range(2 * HPC)]                  # Q heads 0-3, K heads 0-3
        v_sb = [persist.tile([P, DPC], BF16, tag=f"v_{st}", name=f"v_{st}")
                for st in range(S // P)]
        ident_sb = persist.tile([P, P], BF16, tag="ident")
        bqk_sb = persist.tile([P, 2 * HPC], F32, tag="bqk")
        bexp_sb = persist.tile([P, HPC * QB], F32, tag="bexp")
        bv_bc = persist.tile([P, DPC], F32, tag="bv_bc")
        ones_all = persist.tile([P, P], BF16, tag="ones3")
        alsp_a = persist.tile([P, S], BF16, tag="alsp_a")
        alsp_b = persist.tile([3, S], BF16, tag="alsp_b")
        ones3 = [ones_all[32 * h:32 * h + 3, :] for h in range(3)] + \
            [ones_all[0:3, :]]
        alsp_sb = [alsp_a[32 * h:32 * h + 3, :] for h in range(3)] + [alsp_b]

        nc.sync.dma_start(out=ident_sb, in_=ident_t[:, :])
        nc.sync.dma_start(out=bqk_sb, in_=bqk_t[:, :])
        nc.sync.dma_start(out=bexp_sb, in_=bexp_t[:, :])
        nc.vector.memset(ones_all, 1.0)
        for h in range(HPC):
            nc.sync.dma_start(out=alsp_sb[h], in_=alsp_t[h])
        nc.gpsimd.dma_start(
            out=bv_bc, in_=bass.AP(tensor=bv_t, offset=0, ap=[[0, P], [1, DPC]]))

        # ---------------- phase 1: QKV projection ----------------
        with ExitStack() as ph1:
            wq_pool = ph1.enter_context(tc.tile_pool(name="wq", bufs=1))
            hid_pool = ph1.enter_context(tc.tile_pool(name="hid", bufs=2))
            psqk = ph1.enter_context(
                tc.tile_pool(name="psqk", bufs=4, space="PSUM"))
            psv = ph1.enter_context(
                tc.tile_pool(name="psv", bufs=2, space="PSUM"))

            wqk_sb = [wq_pool.tile([P, KT * P], BF16, tag=f"wqk_{f}",
                                   name=f"wqk_{f}") for f in range(2 * HPC)]
            wv_sb = wq_pool.tile([P, KT * DPC], BF16, tag="wv")
            hps = [hid_pool.tile([P, KT * SCW], BF16, tag="hp", name=f"hp_{c}")
                   for c in range(NCH)]
            # first hidden chunk and first weight first, so compute starts fast
            nc.sync.dma_start(out=hps[0], in_=hpk[0])
            for f in range(2 * HPC):
                nc.sync.dma_start(out=wqk_sb[f], in_=wqk[f])
            nc.sync.dma_start(out=wv_sb, in_=wv[:, :])

            for c in range(NCH):
                hp = hps[c]
                if c + 1 < NCH:
                    nc.sync.dma_start(out=hps[c + 1], in_=hpk[c + 1])
                if c == 1:
                    # attention-phase constants; emitted here so they queue
                    # behind the first hidden chunks, not ahead of them
                    nc.sync.dma_start(out=ident_sb, in_=ident_t[:, :])
                    nc.sync.dma_start(out=bexp_sb, in_=bexp_t[:, :])
                    for qb in range(QB):
                        nc.sync.dma_start(out=trit_sb[qb], in_=trit_t[qb])
                    for h in range(HPC):
                        nc.sync.dma_start(out=alsp_sb[h], in_=alsp_t[h])
                for f in range(2 * HPC):
                    ps = psqk.tile([P, SCW], F32, tag="psqk")
                    for j in range(KT):
                        nc.tensor.matmul(
                            ps, wqk_sb[f][:, j * P:(j + 1) * P],
                            hp[:, j * SCW:(j + 1) * SCW],
                            start=(j == 0), stop=(j == KT - 1))
                    nc.scalar.activation(
                        out=qk_sb[f][:, c * SCW:(c + 1) * SCW], in_=ps,
                        func=Ident, bias=bqk_sb[:, f:f + 1], scale=1.0)
                for st2 in range(SCW // P):
                    st = c * (SCW // P) + st2
                    ps = psv.tile([P, DPC], F32, tag="psv")
                    for j in range(KT):
                        nc.tensor.matmul(
                            ps, hp[:, j * SCW + st2 * P: j * SCW + (st2 + 1) * P],
                            wv_sb[:, j * DPC:(j + 1) * DPC],
                            start=(j == 0), stop=(j == KT - 1))
                    nc.vector.tensor_add(out=v_sb[st], in0=ps, in1=bv_bc)

        # ---------------- phases 2+3: attention + dense ----------------
        with ExitStack() as ph2:
            pr_pool = ph2.enter_context(tc.tile_pool(name="prow", bufs=9))
            pq_pool = ph2.enter_context(tc.tile_pool(name="pquad", bufs=9))
            sm_pool = ph2.enter_context(tc.tile_pool(name="small", bufs=8))
            wd_pool = ph2.enter_context(tc.tile_pool(name="wd", bufs=1))
            st_pool = ph2.enter_context(tc.tile_pool(name="ostage", bufs=2))
            ps_sc = ph2.enter_context(
                tc.tile_pool(name="ps_sc", bufs=3, space="PSUM"))
            ps_cx = ph2.enter_context(
                tc.tile_pool(name="ps_cx", bufs=1, space="PSUM"))
            ps_d = ph2.enter_context(
                tc.tile_pool(name="ps_d", bufs=2, space="PSUM"))
            if not USE_DMA_T:
                ps_st = ph2.enter_context(
                    tc.tile_pool(name="ps_st", bufs=2, space="PSUM"))

            trit_pool = ph2.enter_context(tc.tile_pool(name="trit", bufs=1))
            trit_sb = [trit_pool.tile([P, P], BF16, tag=f"trit_{qb}",
                                      name=f"trit_{qb}") for qb in range(QB)]
            for qb in range(QB):
                nc.sync.dma_start(out=trit_sb[qb], in_=trit_t[qb])
            wdp_sb = wd_pool.tile([P, HPC * H], BF16, tag="wdp")
            nc.sync.dma_start(out=wdp_sb, in_=wdp_t[:, :])

            prow_t = {}   # (h, qb) -> prob row tile
            quads = {}    # (h, qg) -> list of quad tiles [P, 4, 512]

            def chain(h, qb):
                """scores (+alibi, -b_q, mask) in PSUM -> exp -> normalized
                bf16 prob row."""
                kN = kNq[qb]
                lo0 = (kLoT[h][qb] * P) // 512 * 512
                nt = (kN - lo0 + 511) // 512
                prow = pr_pool.tile([P, S], BF16, tag="prow",
                                    name=f"prow_{h}_{qb}")
                prow_t[(h, qb)] = prow
                strip = sm_pool.tile([P, 4], F32, tag="strip")
                rinv = sm_pool.tile([P, 1], F32, tag="rinv")
                nb = bexp_sb[:, h * QB + qb: h * QB + qb + 1]
                qst = qk_sb[h][:, qb * P:(qb + 1) * P]
                for ti in range(nt):
                    lo = lo0 + 512 * ti
                    N = min(512, kN - lo)
                    ps = ps_sc.tile([P, 512], F32, tag="ps_sc")
                    sl = ps[:, :N]
                    diag = (lo + N == kN)
                    nc.tensor.matmul(
                        sl, qst, qk_sb[HPC + h][:, lo:lo + N],
                        start=True, stop=False)
                    nc.tensor.matmul(
                        sl, ones3[h],
                        alsp_sb[h][:, lo:lo + N],
                        start=False, stop=not diag)
                    if diag:
                        nc.tensor.matmul(
                            ps[:, N - P:N],
                            trit_sb[qb], ident_sb,
                            start=False, stop=True)
                    nc.scalar.activation(
                        out=prow[:, lo:lo + N], in_=sl,
                        func=Exp, bias=nb, scale=1.0,
                        accum_out=strip[:, ti:ti + 1])
                if nt > 1:
                    tot = sm_pool.tile([P, 1], F32, tag="tot")
                    nc.vector.tensor_reduce(
                        out=tot, in_=strip[:, :nt], op=ADD,
                        axis=mybir.AxisListType.X)
                else:
                    tot = strip[:, 0:1]
                nc.vector.reciprocal(out=rinv, in_=tot)
                nc.vector.tensor_scalar_mul(
                    out=prow[:, lo0:kN], in0=prow[:, lo0:kN], scalar1=rinv)

            def transposes(h, qb):
                """prow(h, qb) -> key-major quad slices."""
                kN = kNq[qb]
                qg, qbl = qb // 4, qb % 4
                if qbl == 0:
                    ntile_g = kNq[4 * qg + 3] // P
                    a0 = kLoT[h][4 * qg] // 4
                    quads[(h, qg)] = {
                        a: pq_pool.tile([P, 4, 512], BF16, tag="pquad",
                                        name=f"pq_{h}_{qg}_{a}")
                        for a in range(a0, (ntile_g + 3) // 4)}
                prow = prow_t.pop((h, qb))
                qlist = quads[(h, qg)]
                ntile = kN // P
                t0 = kLoT[h][qb]
                if USE_DMA_T:
                    for t in range(t0, ntile):
                        nc.sync.dma_start(
                            out=qlist[t // 4][:, t % 4, qbl * P:(qbl + 1) * P],
                            in_=prow[:, t * P:(t + 1) * P], transpose=True)
                else:
                    t = t0
                    while t < ntile:
                        t = (t // 4) * 4          # align to quad boundary
                        lo_t = max(t, t0)
                        cnt = min(8, ntile - t)
                        stg = ps_st.tile([P, 8, P], BF16, tag="stg")
                        for i in range(lo_t - t, cnt):
                            nc.tensor.transpose(
                                stg[:, i, :], prow[:, (t + i) * P:(t + i + 1) * P],
                                ident_sb)
                        for half in range((cnt + 3) // 4):
                            i0 = max(4 * half, lo_t - t)
                            i1 = min(4 * half + 4, cnt)
                            if i0 >= i1:
                                continue
                            nc.vector.tensor_copy(
                                out=qlist[t // 4 + half][:, i0 - 4 * half:
                                                         i1 - 4 * half,
                                                         qbl * P:(qbl + 1) * P],
                                in_=stg[:, i0:i1, :])
                        t += cnt

            def pv(h, qg):
                kns = [kNq[4 * qg + i] for i in range(4)]
                t0s = [kLoT[h][4 * qg + i] for i in range(4)]
                ntile = kns[3] // P
                tiles_per = [k // P for k in kns]
                qlist = quads.pop((h, qg))
                cps = ps_cx.tile([P, 512], F32, tag="ps_cx")
                nc.vector.memset(cps, 0.0)
                for t in range(t0s[0], ntile):
                    cov = [i for i in range(4)
                           if tiles_per[i] > t and t0s[i] <= t]
                    if not cov:
                        continue
                    ilo, ihi = cov[0], cov[-1]
                    nc.tensor.matmul(
                        cps[:, ilo * P:(ihi + 1) * P],
                        v_sb[t][:, h * P:(h + 1) * P],
                        qlist[t // 4][:, t % 4, ilo * P:(ihi + 1) * P],
                        start=False, stop=(t == ntile - 1),
                        skip_group_check=True)
                nc.scalar.copy(
                    out=ctx_sb[h][:, qg * 512:(qg + 1) * 512], in_=cps)

            def dense_qb(qb):
                    stage = st_pool.tile([P, H], BF16, tag="ostage")
                    for oc in range(8):
                        ps = ps_d.tile([P, 512], F32, tag="ps_d")
                        for hh in range(HPC):
                            nc.tensor.matmul(
                                ps, ctx_sb[hh][:, qb * P:(qb + 1) * P],
                                wdp_sb[:, hh * H + oc * 512:
                                       hh * H + (oc + 1) * 512],
                                start=(hh == 0), stop=(hh == HPC - 1))
                        if oc % 2 == 0:
                            nc.vector.tensor_copy(
                                out=stage[:, oc * 512:(oc + 1) * 512], in_=ps)
                        else:
                            nc.scalar.copy(
                                out=stage[:, oc * 512:(oc + 1) * 512], in_=ps)
                    nc.sync.dma_start(
                        out=out_t[qb * P:(qb + 1) * P, :], in_=stage)

            # software-pipelined emission over 16 (qg, h) units, head-major
            # within each query group. Unit-level lag: unit u's four chains
            # are interleaved with unit u-1's transposes; u-1's PV closes at
            # qbl 3. Dense query blocks are spread one-or-two per unit as
            # their query group's ctx completes.
            units = [(qg, h) for qg in range(4) for h in range(HPC)]
            dq = []        # dense qbs ready to emit
            for u, (qg, h) in enumerate(units):
                prev = units[u - 1] if u > 0 else None
                for qbl in range(4):
                    chain(h, 4 * qg + qbl)
                    if prev:
                        transposes(prev[1], 4 * prev[0] + qbl)
                    if qbl == 3 and prev:
                        pv(prev[1], prev[0])
                        if prev[1] == HPC - 1:
                            dq.extend(4 * prev[0] + i for i in range(4))
                for _ in range(2):
                    if dq:
                        dense_qb(dq.pop(0))
            prev = units[-1]
            for qbl in range(4):
                transposes(prev[1], 4 * prev[0] + qbl)
            pv(prev[1], prev[0])
            dq.extend(4 * prev[0] + i for i in range(4))
            for qb in dq:
                dense_qb(qb)

    nc.compile()
    return nc


def _host_prep(hidden_states, alibi, attention_mask, w_qkv, b_qkv, w_dense):
    """Returns (kNq, in_maps) for the 8 cores."""
    hidden = np.asarray(hidden_states, np.float32).reshape(S, H)
    mask = np.asarray(attention_mask).reshape(S, S)
    alibi = np.asarray(alibi, np.float32).reshape(NH, S)
    w_qkv = np.asarray(w_qkv, np.float32)
    b_qkv = np.asarray(b_qkv, np.float32)
    w_dense = np.asarray(w_dense, np.float32)

    allowed = ~mask
    assert allowed.any(axis=1).all(), "fully-masked row"
    limit = S - np.argmax(allowed[:, ::-1], axis=1)      # last allowed + 1
    recon = np.arange(S)[None, :] >= limit[:, None]
    if not np.array_equal(mask, recon):
        raise NotImplementedError("mask is not suffix-structured")
    kNq = []
    for qb in range(QB):
        lb = limit[qb * P:(qb + 1) * P]
        kN = int(math.ceil(lb.max() / P) * P)
        if lb.min() < kN - P:
            raise NotImplementedError("mask boundary spans >128 cols in block")
        kNq.append(kN)
    if any(kNq[i] > kNq[i + 1] for i in range(QB - 1)):
        raise NotImplementedError("non-monotone key ranges")

    bf = ml_dtypes.bfloat16
    hpk = np.ascontiguousarray(
        hidden.reshape(NCH, SCW, KT, P).transpose(0, 3, 2, 1)
    ).reshape(NCH, P, KT * SCW).astype(bf)
    ident = np.eye(P, dtype=np.float32).astype(bf)
    col = np.arange(S)

    # causal diagonal mask tiles, transposed for use as matmul stationary:
    # trit[qb][k, q] = NEG where key kN-P+k is masked for query q
    trit = np.zeros((QB, P, P), np.float32)
    for qb in range(QB):
        kN = kNq[qb]
        lb = limit[qb * P:(qb + 1) * P]
        cc = col[kN - P:kN]
        trit[qb] = np.where(cc[:, None] >= lb[None, :], NEG, 0.0)
    trit = trit.astype(bf)

    wr = w_qkv.reshape(NH, 3, HD, H)
    br = b_qkv.reshape(NH, 3, HD)

    in_maps = []
    all_kLoT = None
    for c in range(NCORES):
        heads = [c + NCORES * j for j in range(HPC)]
        hs = np.asarray(heads)
        Wq = wr[hs, 0].reshape(DPC, H) * INV_NORM
        Wk = wr[hs, 1].reshape(DPC, H)
        Wv = wr[hs, 2].reshape(DPC, H)
        WQK = np.concatenate([Wq, Wk], axis=0)           # [1024, H]
        wqk_c = np.ascontiguousarray(
            WQK.reshape(2 * HPC, P, KT, P).transpose(0, 3, 2, 1)
        ).reshape(2 * HPC, P, KT * P).astype(bf)
        wv_c = np.ascontiguousarray(
            Wv.reshape(DPC, KT, P).transpose(2, 1, 0)
        ).reshape(P, KT * DPC).astype(bf)
        bq = br[hs, 0].reshape(-1) * INV_NORM
        bk = br[hs, 1].reshape(-1)
        bqk_c = np.ascontiguousarray(
            np.concatenate([bq, bk]).reshape(2 * HPC, P).T)
        bv_c = br[hs, 2].reshape(1, DPC)

        al_c = alibi[hs].astype(np.float32)               # [HPC, S]
        # exact 3-way bf16 split of alibi
        a_hi = al_c.astype(bf).astype(np.float32)
        r1 = al_c - a_hi
        a_mid = r1.astype(bf).astype(np.float32)
        a_lo = (r1 - a_mid).astype(bf)
        alsp_c = np.stack(
            [a_hi.astype(bf), a_mid.astype(bf), a_lo], axis=1)  # [HPC,3,S]

        cmax = np.maximum.accumulate(al_c, axis=1)
        bexp_c = np.zeros((P, HPC * QB), np.float32)
        kLoT_c = []
        for h in range(HPC):
            b_row = cmax[h, limit - 1] + CPAD
            klo_h = []
            for qb in range(QB):
                bexp_c[:, h * QB + qb] = -b_row[qb * P:(qb + 1) * P]
                # keys provably dead (exp underflows for every query in block)
                bmin = b_row[qb * P:(qb + 1) * P].min()
                live = al_c[h] >= (bmin - 112.0)
                k0 = int(np.argmax(live)) if live.any() else 0
                klo_h.append(min(k0 // P, kNq[qb] // P - 1))
            kLoT_c.append(tuple(klo_h))
        kLoT_c = tuple(kLoT_c)
        if all_kLoT is None:
            all_kLoT = kLoT_c
        else:
            # one SPMD program for all cores: take the elementwise min
            all_kLoT = tuple(
                tuple(min(a, b) for a, b in zip(ra, rb))
                for ra, rb in zip(all_kLoT, kLoT_c))
        dcols = np.concatenate(
            [np.arange(g * HD, (g + 1) * HD) for g in heads])
        wdp_c = np.ascontiguousarray(
            w_dense[:, dcols].reshape(H, HPC, P)
            .transpose(2, 1, 0)).reshape(P, HPC * H).astype(bf)
        in_maps.append({
            "hpk": hpk, "wqk": wqk_c, "wv": wv_c, "bqk": bqk_c, "bv": bv_c,
            "alsp": alsp_c, "trit": trit, "bexp": bexp_c, "ident": ident,
            "wdp": wdp_c,
        })
    return (tuple(kNq), all_kLoT), in_maps


def kernel(hidden_states, residual, alibi, attention_mask,
           w_qkv, b_qkv, w_dense, b_dense):
    key, in_maps = _host_prep(hidden_states, alibi, attention_mask,
                              w_qkv, b_qkv, w_dense)
    if key not in _CACHE:
        _CACHE[key] = _build(key)
    nc = _CACHE[key]
    res = run_bass_kernel_spmd(nc, in_maps, list(range(NCORES)))
    acc = res.results[0]["out_part"].astype(np.float32)
    for c in range(1, NCORES):
        acc += res.results[c]["out_part"].astype(np.float32)
    out = acc + np.asarray(b_dense, np.float32)[None, :]
    out = out + np.asarray(residual, np.float32).reshape(S, H)
    return out.reshape(B, S, H).astype(np.float32)


# revision 3
# speedup vs baseline: 1.0050x; 1.0050x over previous
"""BloomAttention (B=1, S=2048, H=4096, NH=32) on 8 Trainium2 cores — v3.

Head-parallel TP (4 heads/core), all matmul operands bf16 (fp32 PSUM accum).
 - QKV: PSUM-resident accumulation over the full 4096 contraction per
   256-wide seq chunk (no SBUF/DVE accumulate passes).
 - scores: QK matmul + a 3-row aux matmul (ones3 x [alibi_hi;mid;lo] — exact
   3-way bf16 split of alibi) + a tri^T x identity matmul for the causal
   diagonal block. Everything lands in PSUM; no DVE score pass at all.
 - softmax: exp straight from PSUM on the scalar engine with a
   host-precomputed per-query upper bound as bias (replaces the row max;
   the shift cancels in normalization), accum_out gives row sums.
 - probs normalized by 1/sum (DVE), transposed to key-major via DMA X-bar
   transposes (USE_DMA_T) or PE transposes + copies (fallback).
 - dense: row-parallel partials written bf16; host sums cores+bias+residual.
"""
import math
import numpy as np
from contextlib import ExitStack

import ml_dtypes

import concourse.bacc as bacc
import concourse.bass as bass
import concourse.mybir as mybir
import concourse.tile as tile
from concourse.bass_utils import run_bass_kernel_spmd

B, S, H, NH = 1, 2048, 4096, 32
HD = H // NH            # 128
NCORES = 8
HPC = NH // NCORES      # 4 heads per core
DPC = HPC * HD          # 512
INV_NORM = 1.0 / math.sqrt(HD)
NEG = -1.0e30
CPAD = 15.0             # slack above max alibi in b_q
P = 128
QB = S // P             # 16 query blocks
NCH = 8                 # seq chunks in phase 1
SCW = S // NCH          # 256 seq chunk width
KT = H // P             # 32 contraction tiles
F32 = mybir.dt.float32
BF16 = mybir.dt.bfloat16
ADD = mybir.AluOpType.add

USE_DMA_T = False        # DMA X-bar transposes vs PE transposes

_CACHE = {}


def _build(key):
    kNq, kLoT = key
    nc = bacc.Bacc("TRN2", target_bir_lowering=False, debug=False,
                   num_devices=NCORES)

    hpk = nc.dram_tensor("hpk", [NCH, P, KT * SCW], BF16, kind="ExternalInput")
    wqk = nc.dram_tensor("wqk", [2 * HPC, P, KT * P], BF16, kind="ExternalInput")
    wv = nc.dram_tensor("wv", [P, KT * DPC], BF16, kind="ExternalInput")
    bqk_t = nc.dram_tensor("bqk", [P, 2 * HPC], F32, kind="ExternalInput")
    bv_t = nc.dram_tensor("bv", [1, DPC], F32, kind="ExternalInput")
    alsp_t = nc.dram_tensor("alsp", [HPC, 3, S], BF16, kind="ExternalInput")
    trit_t = nc.dram_tensor("trit", [QB, P, P], BF16, kind="ExternalInput")
    bexp_t = nc.dram_tensor("bexp", [P, HPC * QB], F32, kind="ExternalInput")
    ident_t = nc.dram_tensor("ident", [P, P], BF16, kind="ExternalInput")
    wdp_t = nc.dram_tensor("wdp", [P, HPC * H], BF16, kind="ExternalInput")
    out_t = nc.dram_tensor("out_part", [S, H], BF16, kind="ExternalOutput")

    Ident = mybir.ActivationFunctionType.Identity
    Exp = mybir.ActivationFunctionType.Exp

    with tile.TileContext(nc) as tc, ExitStack() as top:
        persist = top.enter_context(tc.tile_pool(name="persist", bufs=1))
        qk_sb = [persist.tile([P, S], BF16, tag=f"qk_{f}", name=f"qk_{f}")
                 for f in range(2 * HPC)]                  # Q heads 0-3, K heads 0-3
        v_sb = [persist.tile([P, DPC], BF16, tag=f"v_{st}", name=f"v_{st}")
                for st in range(S // P)]
        ident_sb = persist.tile([P, P], BF16, tag="ident")
        bqk_sb = persist.tile([P, 2 * HPC], F32, tag="bqk")
        bexp_sb = persist.tile([P, HPC * QB], F32, tag="bexp")
        bv_bc = persist.tile([P, DPC], F32, tag="bv_bc")
        ones_all = persist.tile([P, P], BF16, tag="ones3")
        alsp_a = persist.tile([P, S], BF16, tag="alsp_a")
        alsp_b = persist.tile([3, S], BF16, tag="alsp_b")
        ones3 = [ones_all[32 * h:32 * h + 3, :] for h in range(3)] + \
            [ones_all[0:3, :]]
        alsp_sb = [alsp_a[32 * h:32 * h + 3, :] for h in range(3)] + [alsp_b]

        nc.sync.dma_start(out=ident_sb, in_=ident_t[:, :])
        nc.sync.dma_start(out=bqk_sb, in_=bqk_t[:, :])
        nc.sync.dma_start(out=bexp_sb, in_=bexp_t[:, :])
        nc.vector.memset(ones_all, 1.0)
        for h in range(HPC):
            nc.sync.dma_start(out=alsp_sb[h], in_=alsp_t[h])
        nc.gpsimd.dma_start(
            out=bv_bc, in_=bass.AP(tensor=bv_t, offset=0, ap=[[0, P], [1, DPC]]))

        # ---------------- phase 1: QKV projection ----------------
        with ExitStack() as ph1:
            wq_pool = ph1.enter_context(tc.tile_pool(name="wq", bufs=1))
            hid_pool = ph1.enter_context(tc.tile_pool(name="hid", bufs=2))
            psqk = ph1.enter_context(
                tc.tile_pool(name="psqk", bufs=4, space="PSUM"))
            psv = ph1.enter_context(
                tc.tile_pool(name="psv", bufs=2, space="PSUM"))

            wqk_sb = [wq_pool.tile([P, KT * P], BF16, tag=f"wqk_{f}",
                                   name=f"wqk_{f}") for f in range(2 * HPC)]
            wv_sb = wq_pool.tile([P, KT * DPC], BF16, tag="wv")
            hps = [hid_pool.tile([P, KT * SCW], BF16, tag="hp", name=f"hp_{c}")
                   for c in range(NCH)]
            # first hidden chunk and first weight first, so compute starts fast
            nc.sync.dma_start(out=hps[0], in_=hpk[0])
            for f in range(2 * HPC):
                nc.sync.dma_start(out=wqk_sb[f], in_=wqk[f])
            nc.sync.dma_start(out=wv_sb, in_=wv[:, :])

            for c in range(NCH):
                hp = hps[c]
                if c + 1 < NCH:
                    nc.sync.dma_start(out=hps[c + 1], in_=hpk[c + 1])
                if c == 1:
                    # attention-phase constants; emitted here so they queue
                    # behind the first hidden chunks, not ahead of them
                    nc.sync.dma_start(out=ident_sb, in_=ident_t[:, :])
                    nc.sync.dma_start(out=bexp_sb, in_=bexp_t[:, :])
                    for qb in range(QB):
                        nc.sync.dma_start(out=trit_sb[qb], in_=trit_t[qb])
                    for h in range(HPC):
                        nc.sync.dma_start(out=alsp_sb[h], in_=alsp_t[h])
                for f in range(2 * HPC):
                    ps = psqk.tile([P, SCW], F32, tag="psqk")
                    for j in range(KT):
                        nc.tensor.matmul(
                            ps, wqk_sb[f][:, j * P:(j + 1) * P],
                            hp[:, j * SCW:(j + 1) * SCW],
                            start=(j == 0), stop=(j == KT - 1))
                    nc.scalar.activation(
                        out=qk_sb[f][:, c * SCW:(c + 1) * SCW], in_=ps,
                        func=Ident, bias=bqk_sb[:, f:f + 1], scale=1.0)
                for st2 in range(SCW // P):
                    st = c * (SCW // P) + st2
                    ps = psv.tile([P, DPC], F32, tag="psv")
                    for j in range(KT):
                        nc.tensor.matmul(
                            ps, hp[:, j * SCW + st2 * P: j * SCW + (st2 + 1) * P],
                            wv_sb[:, j * DPC:(j + 1) * DPC],
                            start=(j == 0), stop=(j == KT - 1))
                    nc.vector.tensor_add(out=v_sb[st], in0=ps, in1=bv_bc)

        # ---------------- phases 2+3: attention + dense ----------------
        with ExitStack() as ph2:
            pr_pool = ph2.enter_context(tc.tile_pool(name="prow", bufs=9))
            pq_pool = ph2.enter_context(tc.tile_pool(name="pquad", bufs=9))
            sm_pool = ph2.enter_context(tc.tile_pool(name="small", bufs=8))
            wd_pool = ph2.enter_context(tc.tile_pool(name="wd", bufs=1))
            st_pool = ph2.enter_context(tc.tile_pool(name="ostage", bufs=2))
            ps_sc = ph2.enter_context(
                tc.tile_pool(name="ps_sc", bufs=3, space="PSUM"))
            ps_cx = ph2.enter_context(
                tc.tile_pool(name="ps_cx", bufs=1, space="PSUM"))
            ps_d = ph2.enter_context(
                tc.tile_pool(name="ps_d", bufs=2, space="PSUM"))
            if not USE_DMA_T:
                ps_st = ph2.enter_context(
                    tc.tile_pool(name="ps_st", bufs=2, space="PSUM"))

            trit_pool = ph2.enter_context(tc.tile_pool(name="trit", bufs=1))
            trit_sb = [trit_pool.tile([P, P], BF16, tag=f"trit_{qb}",
                                      name=f"trit_{qb}") for qb in range(QB)]
            for qb in range(QB):
                nc.sync.dma_start(out=trit_sb[qb], in_=trit_t[qb])
            wdp_sb = wd_pool.tile([P, HPC * H], BF16, tag="wdp")
            nc.sync.dma_start(out=wdp_sb, in_=wdp_t[:, :])

            prow_t = {}   # (h, qb) -> prob row tile
            quads = {}    # (h, qg) -> list of quad tiles [P, 4, 512]

            def chain(h, qb):
                """scores (+alibi, -b_q, mask) in PSUM -> exp -> normalized
                bf16 prob row."""
                kN = kNq[qb]
                lo0 = (kLoT[h][qb] * P) // 512 * 512
                nt = (kN - lo0 + 511) // 512
                prow = pr_pool.tile([P, S], BF16, tag="prow",
                                    name=f"prow_{h}_{qb}")
                prow_t[(h, qb)] = prow
                strip = sm_pool.tile([P, 4], F32, tag="strip")
                rinv = sm_pool.tile([P, 1], F32, tag="rinv")
                nb = bexp_sb[:, h * QB + qb: h * QB + qb + 1]
                qst = qk_sb[h][:, qb * P:(qb + 1) * P]
                for ti in range(nt):
                    lo = lo0 + 512 * ti
                    N = min(512, kN - lo)
                    ps = ps_sc.tile([P, 512], F32, tag="ps_sc")
                    sl = ps[:, :N]
                    diag = (lo + N == kN)
                    nc.tensor.matmul(
                        sl, qst, qk_sb[HPC + h][:, lo:lo + N],
                        start=True, stop=False)
                    nc.tensor.matmul(
                        sl, ones3[h],
                        alsp_sb[h][:, lo:lo + N],
                        start=False, stop=not diag)
                    if diag:
                        nc.tensor.matmul(
                            ps[:, N - P:N],
                            trit_sb[qb], ident_sb,
                            start=False, stop=True)
                    nc.scalar.activation(
                        out=prow[:, lo:lo + N], in_=sl,
                        func=Exp, bias=nb, scale=1.0,
                        accum_out=strip[:, ti:ti + 1])
                if nt > 1:
                    tot = sm_pool.tile([P, 1], F32, tag="tot")
                    nc.vector.tensor_reduce(
                        out=tot, in_=strip[:, :nt], op=ADD,
                        axis=mybir.AxisListType.X)
                else:
                    tot = strip[:, 0:1]
                nc.vector.reciprocal(out=rinv, in_=tot)
                nc.vector.tensor_scalar_mul(
                    out=prow[:, lo0:kN], in0=prow[:, lo0:kN], scalar1=rinv)

            def transposes(h, qb):
                """prow(h, qb) -> key-major quad slices."""
                kN = kNq[qb]
                qg, qbl = qb // 4, qb % 4
                if qbl == 0:
                    ntile_g = kNq[4 * qg + 3] // P
                    a0 = kLoT[h][4 * qg] // 4
                    quads[(h, qg)] = {
                        a: pq_pool.tile([P, 4, 512], BF16, tag="pquad",
                                        name=f"pq_{h}_{qg}_{a}")
                        for a in range(a0, (ntile_g + 3) // 4)}
                prow = prow_t.pop((h, qb))
                qlist = quads[(h, qg)]
                ntile = kN // P
                t0 = kLoT[h][qb]
                if USE_DMA_T:
                    for t in range(t0, ntile):
                        nc.sync.dma_start(
                            out=qlist[t // 4][:, t % 4, qbl * P:(qbl + 1) * P],
                            in_=prow[:, t * P:(t + 1) * P], transpose=True)
                else:
                    t = t0
                    while t < ntile:
                        t = (t // 4) * 4          # align to quad boundary
                        lo_t = max(t, t0)
                        cnt = min(8, ntile - t)
                        stg = ps_st.tile([P, 8, P], BF16, tag="stg")
                        for i in range(lo_t - t, cnt):
                            nc.tensor.transpose(
                                stg[:, i, :], prow[:, (t + i) * P:(t + i + 1) * P],
                                ident_sb)
                        for half in range((cnt + 3) // 4):
                            i0 = max(4 * half, lo_t - t)
                            i1 = min(4 * half + 4, cnt)
                            if i0 >= i1:
                                continue
                            nc.vector.tensor_copy(
                                out=qlist[t // 4 + half][:, i0 - 4 * half:
                                                         i1 - 4 * half,
                                                         qbl * P:(qbl + 1) * P],
                                in_=stg[:, i0:i1, :])
                        t += cnt

            def pv(h, qg):
                kns = [kNq[4 * qg + i] for i in range(4)]
                t0s = [kLoT[h][4 * qg + i] for i in range(4)]
                ntile = kns[3] // P
                tiles_per = [k // P for k in kns]
                qlist = quads.pop((h, qg))
                cps = ps_cx.tile([P, 512], F32, tag="ps_cx")
                nc.vector.memset(cps, 0.0)
                for t in range(t0s[0], ntile):
                    cov = [i for i in range(4)
                           if tiles_per[i] > t and t0s[i] <= t]
                    if not cov:
                        continue
                    ilo, ihi = cov[0], cov[-1]
                    nc.tensor.matmul(
                        cps[:, ilo * P:(ihi + 1) * P],
                        v_sb[t][:, h * P:(h + 1) * P],
                        qlist[t // 4][:, t % 4, ilo * P:(ihi + 1) * P],
                        start=False, stop=(t == ntile - 1),
                        skip_group_check=True)
                nc.scalar.copy(
                    out=ctx_sb[h][:, qg * 512:(qg + 1) * 512], in_=cps)

            def dense_qb(qb):
                    stage = st_pool.tile([P, H], BF16, tag="ostage")
                    for oc in range(8):
                        ps = ps_d.tile([P, 512], F32, tag="ps_d")
                        for hh in range(HPC):
                            nc.tensor.matmul(
                                ps, ctx_sb[hh][:, qb * P:(qb + 1) * P],
                                wdp_sb[:, hh * H + oc * 512:
                                       hh * H + (oc + 1) * 512],
                                start=(hh == 0), stop=(hh == HPC - 1))
                        if oc % 2 == 0:
                            nc.vector.tensor_copy(
                                out=stage[:, oc * 512:(oc + 1) * 512], in_=ps)
                        else:
                            nc.scalar.copy(
                                out=stage[:, oc * 512:(oc + 1) * 512], in_=ps)
                    nc.sync.dma_start(
                        out=out_t[qb * P:(qb + 1) * P, :], in_=stage)

            # software-pipelined emission over 16 (qg, h) units, head-major
            # within each query group. Unit-level lag: unit u's four chains
            # are interleaved with unit u-1's transposes; u-1's PV closes at
            # qbl 3. Dense query blocks are spread one-or-two per unit as
            # their query group's ctx completes.
            units = [(qg, h) for qg in range(4) for h in range(HPC)]
            dq = []        # dense qbs ready to emit
            for u, (qg, h) in enumerate(units):
                prev = units[u - 1] if u > 0 else None
                for qbl in range(4):
                    chain(h, 4 * qg + qbl)
                    if prev:
                        transposes(prev[1], 4 * prev[0] + qbl)
                    if qbl == 3 and prev:
                        pv(prev[1], prev[0])
                        if prev[1] == HPC - 1:
                            dq.extend(4 * prev[0] + i for i in range(4))
                for _ in range(2 if len(dq) > 4 else 1):
                    if dq:
                        dense_qb(dq.pop(0))
            prev = units[-1]
            for qbl in range(4):
                transposes(prev[1], 4 * prev[0] + qbl)
            pv(prev[1], prev[0])
            dq.extend(4 * prev[0] + i for i in range(4))
            for qb in dq:
                dense_qb(qb)

    nc.compile()
    return nc


def _host_prep(hidden_states, alibi, attention_mask, w_qkv, b_qkv, w_dense):
    """Returns (kNq, in_maps) for the 8 cores."""
    hidden = np.asarray(hidden_states, np.float32).reshape(S, H)
    mask = np.asarray(attention_mask).reshape(S, S)
    alibi = np.asarray(alibi, np.float32).reshape(NH, S)
    w_qkv = np.asarray(w_qkv, np.float32)
    b_qkv = np.asarray(b_qkv, np.float32)
    w_dense = np.asarray(w_dense, np.float32)

    allowed = ~mask
    assert allowed.any(axis=1).all(), "fully-masked row"
    limit = S - np.argmax(allowed[:, ::-1], axis=1)      # last allowed + 1
    recon = np.arange(S)[None, :] >= limit[:, None]
    if not np.array_equal(mask, recon):
        raise NotImplementedError("mask is not suffix-structured")
    kNq = []
    for qb in range(QB):
        lb = limit[qb * P:(qb + 1) * P]
        kN = int(math.ceil(lb.max() / P) * P)
        if lb.min() < kN - P:
            raise NotImplementedError("mask boundary spans >128 cols in block")
        kNq.append(kN)
    if any(kNq[i] > kNq[i + 1] for i in range(QB - 1)):
        raise NotImplementedError("non-monotone key ranges")

    bf = ml_dtypes.bfloat16
    hpk = np.ascontiguousarray(
        hidden.reshape(NCH, SCW, KT, P).transpose(0, 3, 2, 1)
    ).reshape(NCH, P, KT * SCW).astype(bf)
    ident = np.eye(P, dtype=np.float32).astype(bf)
    col = np.arange(S)

    # causal diagonal mask tiles, transposed for use as matmul stationary:
    # trit[qb][k, q] = NEG where key kN-P+k is masked for query q
    trit = np.zeros((QB, P, P), np.float32)
    for qb in range(QB):
        kN = kNq[qb]
        lb = limit[qb * P:(qb + 1) * P]
        cc = col[kN - P:kN]
        trit[qb] = np.where(cc[:, None] >= lb[None, :], NEG, 0.0)
    trit = trit.astype(bf)

    wr = w_qkv.reshape(NH, 3, HD, H)
    br = b_qkv.reshape(NH, 3, HD)

    in_maps = []
    all_kLoT = None
    for c in range(NCORES):
        heads = [c + NCORES * j for j in range(HPC)]
        hs = np.asarray(heads)
        Wq = wr[hs, 0].reshape(DPC, H) * INV_NORM
        Wk = wr[hs, 1].reshape(DPC, H)
        Wv = wr[hs, 2].reshape(DPC, H)
        WQK = np.concatenate([Wq, Wk], axis=0)           # [1024, H]
        wqk_c = np.ascontiguousarray(
            WQK.reshape(2 * HPC, P, KT, P).transpose(0, 3, 2, 1)
        ).reshape(2 * HPC, P, KT * P).astype(bf)
        wv_c = np.ascontiguousarray(
            Wv.reshape(DPC, KT, P).transpose(2, 1, 0)
        ).reshape(P, KT * DPC).astype(bf)
        bq = br[hs, 0].reshape(-1) * INV_NORM
        bk = br[hs, 1].reshape(-1)
        bqk_c = np.ascontiguousarray(
            np.concatenate([bq, bk]).reshape(2 * HPC, P).T)
        bv_c = br[hs, 2].reshape(1, DPC)

        al_c = alibi[hs].astype(np.float32)               # [HPC, S]
        # exact 3-way bf16 split of alibi
        a_hi = al_c.astype(bf).astype(np.float32)
        r1 = al_c - a_hi
        a_mid = r1.astype(bf).astype(np.float32)
        a_lo = (r1 - a_mid).astype(bf)
        alsp_c = np.stack(
            [a_hi.astype(bf), a_mid.astype(bf), a_lo], axis=1)  # [HPC,3,S]

        cmax = np.maximum.accumulate(al_c, axis=1)
        bexp_c = np.zeros((P, HPC * QB), np.float32)
        kLoT_c = []
        for h in range(HPC):
            b_row = cmax[h, limit - 1] + CPAD
            klo_h = []
            for qb in range(QB):
                bexp_c[:, h * QB + qb] = -b_row[qb * P:(qb + 1) * P]
                # keys provably dead (exp underflows for every query in block)
                bmin = b_row[qb * P:(qb + 1) * P].min()
                live = al_c[h] >= (bmin - 112.0)
                k0 = int(np.argmax(live)) if live.any() else 0
                klo_h.append(min(k0 // P, kNq[qb] // P - 1))
            kLoT_c.append(tuple(klo_h))
        kLoT_c = tuple(kLoT_c)
        if all_kLoT is None:
            all_kLoT = kLoT_c
        else:
            # one SPMD program for all cores: take the elementwise min
            all_kLoT = tuple(
                tuple(min(a, b) for a, b in zip(ra, rb))
                for ra, rb in zip(all_kLoT, kLoT_c))
        dcols = np.concatenate(
            [np.arange(g * HD, (g + 1) * HD) for g in heads])
        wdp_c = np.ascontiguousarray(
            w_dense[:, dcols].reshape(H, HPC, P)
            .transpose(2, 1, 0)).reshape(P, HPC * H).astype(bf)
        in_maps.append({
            "hpk": hpk, "wqk": wqk_c, "wv": wv_c, "bqk": bqk_c, "bv": bv_c,
            "alsp": alsp_c, "trit": trit, "bexp": bexp_c, "ident": ident,
            "wdp": wdp_c,
        })
    return (tuple(kNq), all_kLoT), in_maps


def kernel(hidden_states, residual, alibi, attention_mask,
           w_qkv, b_qkv, w_dense, b_dense):
    key, in_maps = _host_prep(hidden_states, alibi, attention_mask,
                              w_qkv, b_qkv, w_dense)
    if key not in _CACHE:
        _CACHE[key] = _build(key)
    nc = _CACHE[key]
    res = run_bass_kernel_spmd(nc, in_maps, list(range(NCORES)))
    acc = res.results[0]["out_part"].astype(np.float32)
    for c in range(1, NCORES):
        acc += res.results[c]["out_part"].astype(np.float32)
    out = acc + np.asarray(b_dense, np.float32)[None, :]
    out = out + np.asarray(residual, np.float32).reshape(S, H)
    return out.reshape(B, S, H).astype(np.float32)


# revision 4
# speedup vs baseline: 1.0253x; 1.0203x over previous
"""BloomAttention (B=1, S=2048, H=4096, NH=32) on 8 Trainium2 cores — v3.

Head-parallel TP (4 heads/core), all matmul operands bf16 (fp32 PSUM accum).
 - QKV: PSUM-resident accumulation over the full 4096 contraction per
   256-wide seq chunk (no SBUF/DVE accumulate passes).
 - scores: QK matmul + a 3-row aux matmul (ones3 x [alibi_hi;mid;lo] — exact
   3-way bf16 split of alibi) + a tri^T x identity matmul for the causal
   diagonal block. Everything lands in PSUM; no DVE score pass at all.
 - softmax: exp straight from PSUM on the scalar engine with a
   host-precomputed per-query upper bound as bias (replaces the row max;
   the shift cancels in normalization), accum_out gives row sums.
 - probs normalized by 1/sum (DVE), transposed to key-major via DMA X-bar
   transposes (USE_DMA_T) or PE transposes + copies (fallback).
 - dense: row-parallel partials written bf16; host sums cores+bias+residual.
"""
import math
import numpy as np
from contextlib import ExitStack

import ml_dtypes

import concourse.bacc as bacc
import concourse.bass as bass
import concourse.mybir as mybir
import concourse.tile as tile
from concourse.bass_utils import run_bass_kernel_spmd

B, S, H, NH = 1, 2048, 4096, 32
HD = H // NH            # 128
NCORES = 8
HPC = NH // NCORES      # 4 heads per core
DPC = HPC * HD          # 512
INV_NORM = 1.0 / math.sqrt(HD)
NEG = -1.0e30
CPAD = 15.0             # slack above max alibi in b_q
P = 128
QB = S // P             # 16 query blocks
NCH = 8                 # seq chunks in phase 1
SCW = S // NCH          # 256 seq chunk width
KT = H // P             # 32 contraction tiles
F32 = mybir.dt.float32
BF16 = mybir.dt.bfloat16
ADD = mybir.AluOpType.add

USE_DMA_T = False        # DMA X-bar transposes vs PE transposes

_CACHE = {}


def _build(key):
    kNq, kLoT = key
    nc = bacc.Bacc("TRN2", target_bir_lowering=False, debug=False,
                   num_devices=NCORES)

    hpk = nc.dram_tensor("hpk", [NCH, P, KT * SCW], BF16, kind="ExternalInput")
    wqk = nc.dram_tensor("wqk", [2 * HPC, P, KT * P], BF16, kind="ExternalInput")
    wv = nc.dram_tensor("wv", [P, KT * DPC], BF16, kind="ExternalInput")
    bqk_t = nc.dram_tensor("bqk", [P, 2 * HPC], F32, kind="ExternalInput")
    bv_t = nc.dram_tensor("bv", [1, DPC], F32, kind="ExternalInput")
    alsp_t = nc.dram_tensor("alsp", [HPC, 3, S], BF16, kind="ExternalInput")
    trit_t = nc.dram_tensor("trit", [QB, P, P], BF16, kind="ExternalInput")
    bexp_t = nc.dram_tensor("bexp", [P, HPC * QB], F32, kind="ExternalInput")
    ident_t = nc.dram_tensor("ident", [P, P], BF16, kind="ExternalInput")
    wdp_t = nc.dram_tensor("wdp", [P, HPC * H], BF16, kind="ExternalInput")
    out_t = nc.dram_tensor("out_part", [S, H], BF16, kind="ExternalOutput")

    Ident = mybir.ActivationFunctionType.Identity
    Exp = mybir.ActivationFunctionType.Exp

    with tile.TileContext(nc) as tc, ExitStack() as top:
        persist = top.enter_context(tc.tile_pool(name="persist", bufs=1))
        qk_sb = [persist.tile([P, S], BF16, tag=f"qk_{f}", name=f"qk_{f}")
                 for f in range(2 * HPC)]                  # Q heads 0-3, K heads 0-3
        v_sb = [persist.tile([P, DPC], BF16, tag=f"v_{st}", name=f"v_{st}")
                for st in range(S // P)]
        ident_sb = persist.tile([P, P], BF16, tag="ident")
        bqk_sb = persist.tile([P, 2 * HPC], F32, tag="bqk")
        bexp_sb = persist.tile([P, HPC * QB], F32, tag="bexp")
        bv_bc = persist.tile([P, DPC], F32, tag="bv_bc")
        ones_all = persist.tile([P, P], BF16, tag="ones3")
        alsp_a = persist.tile([P, S], BF16, tag="alsp_a")
        alsp_b = persist.tile([3, S], BF16, tag="alsp_b")
        ones3 = [ones_all[32 * h:32 * h + 3, :] for h in range(3)] + \
            [ones_all[0:3, :]]
        alsp_sb = [alsp_a[32 * h:32 * h + 3, :] for h in range(3)] + [alsp_b]

        nc.sync.dma_start(out=ident_sb, in_=ident_t[:, :])
        nc.sync.dma_start(out=bqk_sb, in_=bqk_t[:, :])
        nc.sync.dma_start(out=bexp_sb, in_=bexp_t[:, :])
        nc.vector.memset(ones_all, 1.0)
        for h in range(HPC):
            nc.sync.dma_start(out=alsp_sb[h], in_=alsp_t[h])
        nc.gpsimd.dma_start(
            out=bv_bc, in_=bass.AP(tensor=bv_t, offset=0, ap=[[0, P], [1, DPC]]))

        # ---------------- phase 1: QKV projection ----------------
        with ExitStack() as ph1:
            wq_pool = ph1.enter_context(tc.tile_pool(name="wq", bufs=1))
            hid_pool = ph1.enter_context(tc.tile_pool(name="hid", bufs=2))
            psqk = ph1.enter_context(
                tc.tile_pool(name="psqk", bufs=5, space="PSUM"))
            psv = ph1.enter_context(
                tc.tile_pool(name="psv", bufs=3, space="PSUM"))

            wqk_sb = [wq_pool.tile([P, KT * P], BF16, tag=f"wqk_{f}",
                                   name=f"wqk_{f}") for f in range(2 * HPC)]
            wv_sb = wq_pool.tile([P, KT * DPC], BF16, tag="wv")
            hps = [hid_pool.tile([P, KT * SCW], BF16, tag="hp", name=f"hp_{c}")
                   for c in range(NCH)]
            # first hidden chunk and first weight first, so compute starts fast
            nc.sync.dma_start(out=hps[0], in_=hpk[0])
            for f in range(2 * HPC):
                nc.sync.dma_start(out=wqk_sb[f], in_=wqk[f])
            nc.sync.dma_start(out=wv_sb, in_=wv[:, :])

            for c in range(NCH):
                hp = hps[c]
                if c + 1 < NCH:
                    nc.sync.dma_start(out=hps[c + 1], in_=hpk[c + 1])
                if c == 1:
                    # attention-phase constants; emitted here so they queue
                    # behind the first hidden chunks, not ahead of them
                    nc.sync.dma_start(out=ident_sb, in_=ident_t[:, :])
                    nc.sync.dma_start(out=bexp_sb, in_=bexp_t[:, :])
                    for qb in range(QB):
                        nc.sync.dma_start(out=trit_sb[qb], in_=trit_t[qb])
                    for h in range(HPC):
                        nc.sync.dma_start(out=alsp_sb[h], in_=alsp_t[h])
                for f in range(2 * HPC):
                    ps = psqk.tile([P, SCW], F32, tag="psqk")
                    for j in range(KT):
                        nc.tensor.matmul(
                            ps, wqk_sb[f][:, j * P:(j + 1) * P],
                            hp[:, j * SCW:(j + 1) * SCW],
                            start=(j == 0), stop=(j == KT - 1))
                    nc.scalar.activation(
                        out=qk_sb[f][:, c * SCW:(c + 1) * SCW], in_=ps,
                        func=Ident, bias=bqk_sb[:, f:f + 1], scale=1.0)
                for st2 in range(SCW // P):
                    st = c * (SCW // P) + st2
                    ps = psv.tile([P, DPC], F32, tag="psv")
                    for j in range(KT):
                        nc.tensor.matmul(
                            ps, hp[:, j * SCW + st2 * P: j * SCW + (st2 + 1) * P],
                            wv_sb[:, j * DPC:(j + 1) * DPC],
                            start=(j == 0), stop=(j == KT - 1))
                    nc.vector.tensor_add(out=v_sb[st], in0=ps, in1=bv_bc)

        # ---------------- phases 2+3: attention + dense ----------------
        with ExitStack() as ph2:
            pr_pool = ph2.enter_context(tc.tile_pool(name="prow", bufs=9))
            pq_pool = ph2.enter_context(tc.tile_pool(name="pquad", bufs=9))
            sm_pool = ph2.enter_context(tc.tile_pool(name="small", bufs=8))
            wd_pool = ph2.enter_context(tc.tile_pool(name="wd", bufs=1))
            st_pool = ph2.enter_context(tc.tile_pool(name="ostage", bufs=2))
            ps_sc = ph2.enter_context(
                tc.tile_pool(name="ps_sc", bufs=3, space="PSUM"))
            ps_cx = ph2.enter_context(
                tc.tile_pool(name="ps_cx", bufs=1, space="PSUM"))
            ps_d = ph2.enter_context(
                tc.tile_pool(name="ps_d", bufs=2, space="PSUM"))
            if not USE_DMA_T:
                ps_st = ph2.enter_context(
                    tc.tile_pool(name="ps_st", bufs=2, space="PSUM"))

            trit_pool = ph2.enter_context(tc.tile_pool(name="trit", bufs=1))
            trit_sb = [trit_pool.tile([P, P], BF16, tag=f"trit_{qb}",
                                      name=f"trit_{qb}") for qb in range(QB)]
            for qb in range(QB):
                nc.sync.dma_start(out=trit_sb[qb], in_=trit_t[qb])
            wdp_sb = wd_pool.tile([P, HPC * H], BF16, tag="wdp")
            nc.sync.dma_start(out=wdp_sb, in_=wdp_t[:, :])

            prow_t = {}   # (h, qb) -> prob row tile
            quads = {}    # (h, qg) -> list of quad tiles [P, 4, 512]

            def chain(h, qb):
                """scores (+alibi, -b_q, mask) in PSUM -> exp -> normalized
                bf16 prob row."""
                kN = kNq[qb]
                lo0 = (kLoT[h][qb] * P) // 512 * 512
                nt = (kN - lo0 + 511) // 512
                prow = pr_pool.tile([P, S], BF16, tag="prow",
                                    name=f"prow_{h}_{qb}")
                prow_t[(h, qb)] = prow
                strip = sm_pool.tile([P, 4], F32, tag="strip")
                rinv = sm_pool.tile([P, 1], F32, tag="rinv")
                nb = bexp_sb[:, h * QB + qb: h * QB + qb + 1]
                qst = qk_sb[h][:, qb * P:(qb + 1) * P]
                for ti in range(nt):
                    lo = lo0 + 512 * ti
                    N = min(512, kN - lo)
                    ps = ps_sc.tile([P, 512], F32, tag="ps_sc")
                    sl = ps[:, :N]
                    diag = (lo + N == kN)
                    nc.tensor.matmul(
                        sl, qst, qk_sb[HPC + h][:, lo:lo + N],
                        start=True, stop=False)
                    nc.tensor.matmul(
                        sl, ones3[h],
                        alsp_sb[h][:, lo:lo + N],
                        start=False, stop=not diag)
                    if diag:
                        nc.tensor.matmul(
                            ps[:, N - P:N],
                            trit_sb[qb], ident_sb,
                            start=False, stop=True)
                    nc.scalar.activation(
                        out=prow[:, lo:lo + N], in_=sl,
                        func=Exp, bias=nb, scale=1.0,
                        accum_out=strip[:, ti:ti + 1])
                if nt > 1:
                    tot = sm_pool.tile([P, 1], F32, tag="tot")
                    nc.vector.tensor_reduce(
                        out=tot, in_=strip[:, :nt], op=ADD,
                        axis=mybir.AxisListType.X)
                else:
                    tot = strip[:, 0:1]
                nc.vector.reciprocal(out=rinv, in_=tot)
                nc.vector.tensor_scalar_mul(
                    out=prow[:, lo0:kN], in0=prow[:, lo0:kN], scalar1=rinv)

            def transposes(h, qb):
                """prow(h, qb) -> key-major quad slices."""
                kN = kNq[qb]
                qg, qbl = qb // 4, qb % 4
                if qbl == 0:
                    ntile_g = kNq[4 * qg + 3] // P
                    a0 = kLoT[h][4 * qg] // 4
                    quads[(h, qg)] = {
                        a: pq_pool.tile([P, 4, 512], BF16, tag="pquad",
                                        name=f"pq_{h}_{qg}_{a}")
                        for a in range(a0, (ntile_g + 3) // 4)}
                prow = prow_t.pop((h, qb))
                qlist = quads[(h, qg)]
                ntile = kN // P
                t0 = kLoT[h][qb]
                if USE_DMA_T:
                    for t in range(t0, ntile):
                        nc.sync.dma_start(
                            out=qlist[t // 4][:, t % 4, qbl * P:(qbl + 1) * P],
                            in_=prow[:, t * P:(t + 1) * P], transpose=True)
                else:
                    t = t0
                    while t < ntile:
                        t = (t // 4) * 4          # align to quad boundary
                        lo_t = max(t, t0)
                        cnt = min(8, ntile - t)
                        stg = ps_st.tile([P, 8, P], BF16, tag="stg")
                        for i in range(lo_t - t, cnt):
                            nc.tensor.transpose(
                                stg[:, i, :], prow[:, (t + i) * P:(t + i + 1) * P],
                                ident_sb)
                        for half in range((cnt + 3) // 4):
                            i0 = max(4 * half, lo_t - t)
                            i1 = min(4 * half + 4, cnt)
                            if i0 >= i1:
                                continue
                            nc.vector.tensor_copy(
                                out=qlist[t // 4 + half][:, i0 - 4 * half:
                                                         i1 - 4 * half,
                                                         qbl * P:(qbl + 1) * P],
                                in_=stg[:, i0:i1, :])
                        t += cnt

            def pv(h, qg):
                kns = [kNq[4 * qg + i] for i in range(4)]
                t0s = [kLoT[h][4 * qg + i] for i in range(4)]
                ntile = kns[3] // P
                tiles_per = [k // P for k in kns]
                qlist = quads.pop((h, qg))
                cps = ps_cx.tile([P, 512], F32, tag="ps_cx")
                nc.vector.memset(cps, 0.0)
                for t in range(t0s[0], ntile):
                    cov = [i for i in range(4)
                           if tiles_per[i] > t and t0s[i] <= t]
                    if not cov:
                        continue
                    ilo, ihi = cov[0], cov[-1]
                    nc.tensor.matmul(
                        cps[:, ilo * P:(ihi + 1) * P],
                        v_sb[t][:, h * P:(h + 1) * P],
                        qlist[t // 4][:, t % 4, ilo * P:(ihi + 1) * P],
                        start=False, stop=(t == ntile - 1),
                        skip_group_check=True)
                nc.vector.tensor_copy(
                    out=ctx_sb[h][:, qg * 512:(qg + 1) * 512], in_=cps)

            def dense_qb(qb, split_dma=False):
                    stage = st_pool.tile([P, H], BF16, tag="ostage")
                    if split_dma:
                        pass
                    for oc in range(8):
                        ps = ps_d.tile([P, 512], F32, tag="ps_d")
                        for hh in range(HPC):
                            nc.tensor.matmul(
                                ps, ctx_sb[hh][:, qb * P:(qb + 1) * P],
                                wdp_sb[:, hh * H + oc * 512:
                                       hh * H + (oc + 1) * 512],
                                start=(hh == 0), stop=(hh == HPC - 1))
                        if oc % 2 == 0:
                            nc.vector.tensor_copy(
                                out=stage[:, oc * 512:(oc + 1) * 512], in_=ps)
                        else:
                            nc.scalar.copy(
                                out=stage[:, oc * 512:(oc + 1) * 512], in_=ps)
                        if split_dma and oc == 3:
                            nc.sync.dma_start(
                                out=out_t[qb * P:(qb + 1) * P, :H // 2],
                                in_=stage[:, :H // 2])
                    if split_dma:
                        nc.sync.dma_start(
                            out=out_t[qb * P:(qb + 1) * P, H // 2:],
                            in_=stage[:, H // 2:])
                    else:
                        nc.sync.dma_start(
                            out=out_t[qb * P:(qb + 1) * P, :], in_=stage)

            # software-pipelined emission over 16 (qg, h) units, head-major
            # within each query group. Unit-level lag: unit u's four chains
            # are interleaved with unit u-1's transposes; u-1's PV closes at
            # qbl 3. Dense query blocks are spread one-or-two per unit as
            # their query group's ctx completes.
            units = [(qg, h) for qg in range(4) for h in range(HPC)]
            dq = []        # dense qbs ready to emit
            for u, (qg, h) in enumerate(units):
                prev = units[u - 1] if u > 0 else None
                for qbl in range(4):
                    chain(h, 4 * qg + qbl)
                    if prev:
                        transposes(prev[1], 4 * prev[0] + qbl)
                    if qbl == 3 and prev:
                        pv(prev[1], prev[0])
                        if prev[1] == HPC - 1:
                            dq.extend(4 * prev[0] + i for i in range(4))
                for _ in range(2 if len(dq) > 4 else 1):
                    if dq:
                        dense_qb(dq.pop(0))
            prev = units[-1]
            for qbl in range(4):
                transposes(prev[1], 4 * prev[0] + qbl)
            pv(prev[1], prev[0])
            dq.extend(4 * prev[0] + i for i in range(4))
            for i, qb in enumerate(dq):
                dense_qb(qb, split_dma=(i == len(dq) - 1))

    nc.compile()
    return nc


def _host_prep(hidden_states, alibi, attention_mask, w_qkv, b_qkv, w_dense):
    """Returns (kNq, in_maps) for the 8 cores."""
    hidden = np.asarray(hidden_states, np.float32).reshape(S, H)
    mask = np.asarray(attention_mask).reshape(S, S)
    alibi = np.asarray(alibi, np.float32).reshape(NH, S)
    w_qkv = np.asarray(w_qkv, np.float32)
    b_qkv = np.asarray(b_qkv, np.float32)
    w_dense = np.asarray(w_dense, np.float32)

    allowed = ~mask
    assert allowed.any(axis=1).all(), "fully-masked row"
    limit = S - np.argmax(allowed[:, ::-1], axis=1)      # last allowed + 1
    recon = np.arange(S)[None, :] >= limit[:, None]
    if not np.array_equal(mask, recon):
        raise NotImplementedError("mask is not suffix-structured")
    kNq = []
    for qb in range(QB):
        lb = limit[qb * P:(qb + 1) * P]
        kN = int(math.ceil(lb.max() / P) * P)
        if lb.min() < kN - P:
            raise NotImplementedError("mask boundary spans >128 cols in block")
        kNq.append(kN)
    if any(kNq[i] > kNq[i + 1] for i in range(QB - 1)):
        raise NotImplementedError("non-monotone key ranges")

    bf = ml_dtypes.bfloat16
    hpk = np.ascontiguousarray(
        hidden.reshape(NCH, SCW, KT, P).transpose(0, 3, 2, 1)
    ).reshape(NCH, P, KT * SCW).astype(bf)
    ident = np.eye(P, dtype=np.float32).astype(bf)
    col = np.arange(S)

    # causal diagonal mask tiles, transposed for use as matmul stationary:
    # trit[qb][k, q] = NEG where key kN-P+k is masked for query q
    trit = np.zeros((QB, P, P), np.float32)
    for qb in range(QB):
        kN = kNq[qb]
        lb = limit[qb * P:(qb + 1) * P]
        cc = col[kN - P:kN]
        trit[qb] = np.where(cc[:, None] >= lb[None, :], NEG, 0.0)
    trit = trit.astype(bf)

    wr = w_qkv.reshape(NH, 3, HD, H)
    br = b_qkv.reshape(NH, 3, HD)

    in_maps = []
    all_kLoT = None
    for c in range(NCORES):
        heads = [c + NCORES * j for j in range(HPC)]
        hs = np.asarray(heads)
        Wq = wr[hs, 0].reshape(DPC, H) * INV_NORM
        Wk = wr[hs, 1].reshape(DPC, H)
        Wv = wr[hs, 2].reshape(DPC, H)
        WQK = np.concatenate([Wq, Wk], axis=0)           # [1024, H]
        wqk_c = np.ascontiguousarray(
            WQK.reshape(2 * HPC, P, KT, P).transpose(0, 3, 2, 1)
        ).reshape(2 * HPC, P, KT * P).astype(bf)
        wv_c = np.ascontiguousarray(
            Wv.reshape(DPC, KT, P).transpose(2, 1, 0)
        ).reshape(P, KT * DPC).astype(bf)
        bq = br[hs, 0].reshape(-1) * INV_NORM
        bk = br[hs, 1].reshape(-1)
        bqk_c = np.ascontiguousarray(
            np.concatenate([bq, bk]).reshape(2 * HPC, P).T)
        bv_c = br[hs, 2].reshape(1, DPC)

        al_c = alibi[hs].astype(np.float32)               # [HPC, S]
        # exact 3-way bf16 split of alibi
        a_hi = al_c.astype(bf).astype(np.float32)
        r1 = al_c - a_hi
        a_mid = r1.astype(bf).astype(np.float32)
        a_lo = (r1 - a_mid).astype(bf)
        alsp_c = np.stack(
            [a_hi.astype(bf), a_mid.astype(bf), a_lo], axis=1)  # [HPC,3,S]

        cmax = np.maximum.accumulate(al_c, axis=1)
        bexp_c = np.zeros((P, HPC * QB), np.float32)
        kLoT_c = []
        for h in range(HPC):
            b_row = cmax[h, limit - 1] + CPAD
            klo_h = []
            for qb in range(QB):
                bexp_c[:, h * QB + qb] = -b_row[qb * P:(qb + 1) * P]
                # keys provably dead (exp underflows for every query in block)
                bmin = b_row[qb * P:(qb + 1) * P].min()
                live = al_c[h] >= (bmin - 112.0)
                k0 = int(np.argmax(live)) if live.any() else 0
                klo_h.append(min(k0 // P, kNq[qb] // P - 1))
            kLoT_c.append(tuple(klo_h))
        kLoT_c = tuple(kLoT_c)
        if all_kLoT is None:
            all_kLoT = kLoT_c
        else:
            # one SPMD program for all cores: take the elementwise min
            all_kLoT = tuple(
                tuple(min(a, b) for a, b in zip(ra, rb))
                for ra, rb in zip(all_kLoT, kLoT_c))
        dcols = np.concatenate(
            [np.arange(g * HD, (g + 1) * HD) for g in heads])
        wdp_c = np.ascontiguousarray(
            w_dense[:, dcols].reshape(H, HPC, P)
            .transpose(2, 1, 0)).reshape(P, HPC * H).astype(bf)
        in_maps.append({
            "hpk": hpk, "wqk": wqk_c, "wv": wv_c, "bqk": bqk_c, "bv": bv_c,
            "alsp": alsp_c, "trit": trit, "bexp": bexp_c, "ident": ident,
            "wdp": wdp_c,
        })
    return (tuple(kNq), all_kLoT), in_maps


def kernel(hidden_states, residual, alibi, attention_mask,
           w_qkv, b_qkv, w_dense, b_dense):
    key, in_maps = _host_prep(hidden_states, alibi, attention_mask,
                              w_qkv, b_qkv, w_dense)
    if key not in _CACHE:
        _CACHE[key] = _build(key)
    nc = _CACHE[key]
    res = run_bass_kernel_spmd(nc, in_maps, list(range(NCORES)))
    acc = res.results[0]["out_part"].astype(np.float32)
    for c in range(1, NCORES):
        acc += res.results[c]["out_part"].astype(np.float32)
    out = acc + np.asarray(b_dense, np.float32)[None, :]
    out = out + np.asarray(residual, np.float32).reshape(S, H)
    return out.reshape(B, S, H).astype(np.float32)


# revision 5
# speedup vs baseline: 1.0532x; 1.0272x over previous
"""BloomAttention (B=1, S=2048, H=4096, NH=32) on 8 Trainium2 cores — v3.

Head-parallel TP (4 heads/core), all matmul operands bf16 (fp32 PSUM accum).
 - QKV: PSUM-resident accumulation over the full 4096 contraction per
   256-wide seq chunk (no SBUF/DVE accumulate passes).
 - scores: QK matmul + a 3-row aux matmul (ones3 x [alibi_hi;mid;lo] — exact
   3-way bf16 split of alibi) + a tri^T x identity matmul for the causal
   diagonal block. Everything lands in PSUM; no DVE score pass at all.
 - softmax: exp straight from PSUM on the scalar engine with a
   host-precomputed per-query upper bound as bias (replaces the row max;
   the shift cancels in normalization), accum_out gives row sums.
 - probs normalized by 1/sum (DVE), transposed to key-major via DMA X-bar
   transposes (USE_DMA_T) or PE transposes + copies (fallback).
 - dense: row-parallel partials written bf16; host sums cores+bias+residual.
"""
import math
import numpy as np
from contextlib import ExitStack

import ml_dtypes

import concourse.bacc as bacc
import concourse.bass as bass
import concourse.mybir as mybir
import concourse.tile as tile
from concourse.bass_utils import run_bass_kernel_spmd

B, S, H, NH = 1, 2048, 4096, 32
HD = H // NH            # 128
NCORES = 8
HPC = NH // NCORES      # 4 heads per core
DPC = HPC * HD          # 512
INV_NORM = 1.0 / math.sqrt(HD)
NEG = -1.0e30
CPAD = 15.0             # slack above max alibi in b_q
P = 128
QB = S // P             # 16 query blocks
NCH = 8                 # seq chunks in phase 1
SCW = S // NCH          # 256 seq chunk width
KT = H // P             # 32 contraction tiles
F32 = mybir.dt.float32
BF16 = mybir.dt.bfloat16
ADD = mybir.AluOpType.add

USE_DMA_T = False        # DMA X-bar transposes vs PE transposes

_CACHE = {}


def _build(key):
    kNq, kLoT = key
    nc = bacc.Bacc("TRN2", target_bir_lowering=False, debug=False,
                   num_devices=NCORES)

    hpk = nc.dram_tensor("hpk", [NCH, P, KT * SCW], BF16, kind="ExternalInput")
    wqk = nc.dram_tensor("wqk", [2 * HPC, P, KT * P], BF16, kind="ExternalInput")
    wv = nc.dram_tensor("wv", [P, KT * DPC], BF16, kind="ExternalInput")
    bqk_t = nc.dram_tensor("bqk", [P, 2 * HPC], F32, kind="ExternalInput")
    bv_t = nc.dram_tensor("bv", [1, DPC], F32, kind="ExternalInput")
    alsp_t = nc.dram_tensor("alsp", [HPC, 3, S], BF16, kind="ExternalInput")
    trit_t = nc.dram_tensor("trit", [QB, P, P], BF16, kind="ExternalInput")
    bexp_t = nc.dram_tensor("bexp", [P, HPC * QB], F32, kind="ExternalInput")
    ident_t = nc.dram_tensor("ident", [P, P], BF16, kind="ExternalInput")
    wdp_t = nc.dram_tensor("wdp", [P, HPC * H], BF16, kind="ExternalInput")
    out_t = nc.dram_tensor("out_part", [S, H], BF16, kind="ExternalOutput")

    Ident = mybir.ActivationFunctionType.Identity
    Exp = mybir.ActivationFunctionType.Exp

    with tile.TileContext(nc) as tc, ExitStack() as top:
        persist = top.enter_context(tc.tile_pool(name="persist", bufs=1))
        qk_sb = [persist.tile([P, S], BF16, tag=f"qk_{f}", name=f"qk_{f}")
                 for f in range(2 * HPC)]                  # Q heads 0-3, K heads 0-3
        v_sb = [persist.tile([P, DPC], BF16, tag=f"v_{st}", name=f"v_{st}")
                for st in range(S // P)]
        ident_sb = persist.tile([P, P], BF16, tag="ident")
        bqk_sb = persist.tile([P, 2 * HPC], F32, tag="bqk")
        bexp_sb = persist.tile([P, HPC * QB], F32, tag="bexp")
        bv_bc = persist.tile([P, DPC], F32, tag="bv_bc")
        ones_all = persist.tile([P, P], BF16, tag="ones3")
        alsp_a = persist.tile([P, S], BF16, tag="alsp_a")
        alsp_b = persist.tile([3, S], BF16, tag="alsp_b")
        ones3 = [ones_all[32 * h:32 * h + 3, :] for h in range(3)] + \
            [ones_all[0:3, :]]
        alsp_sb = [alsp_a[32 * h:32 * h + 3, :] for h in range(3)] + [alsp_b]

        nc.sync.dma_start(out=ident_sb, in_=ident_t[:, :])
        nc.sync.dma_start(out=bqk_sb, in_=bqk_t[:, :])
        nc.sync.dma_start(out=bexp_sb, in_=bexp_t[:, :])
        nc.vector.memset(ones_all, 1.0)
        for h in range(HPC):
            nc.sync.dma_start(out=alsp_sb[h], in_=alsp_t[h])
        nc.gpsimd.dma_start(
            out=bv_bc, in_=bass.AP(tensor=bv_t, offset=0, ap=[[0, P], [1, DPC]]))

        # ---------------- phase 1: QKV projection ----------------
        with ExitStack() as ph1:
            wq_pool = ph1.enter_context(tc.tile_pool(name="wq", bufs=1))
            hid_pool = ph1.enter_context(tc.tile_pool(name="hid", bufs=2))
            psqk = ph1.enter_context(
                tc.tile_pool(name="psqk", bufs=5, space="PSUM"))
            psv = ph1.enter_context(
                tc.tile_pool(name="psv", bufs=3, space="PSUM"))

            wqk_sb = [wq_pool.tile([P, KT * P], BF16, tag=f"wqk_{f}",
                                   name=f"wqk_{f}") for f in range(2 * HPC)]
            wv_sb = wq_pool.tile([P, KT * DPC], BF16, tag="wv")
            hps = [hid_pool.tile([P, KT * SCW], BF16, tag="hp", name=f"hp_{c}")
                   for c in range(NCH)]
            # first hidden chunk and first weight first, so compute starts fast
            nc.sync.dma_start(out=hps[0], in_=hpk[0])
            for f in range(2 * HPC):
                nc.sync.dma_start(out=wqk_sb[f], in_=wqk[f])
            nc.sync.dma_start(out=wv_sb, in_=wv[:, :])

            for c in range(NCH):
                hp = hps[c]
                if c + 1 < NCH:
                    nc.sync.dma_start(out=hps[c + 1], in_=hpk[c + 1])
                if c == 1:
                    # attention-phase constants; emitted here so they queue
                    # behind the first hidden chunks, not ahead of them
                    nc.sync.dma_start(out=ident_sb, in_=ident_t[:, :])
                    nc.sync.dma_start(out=bexp_sb, in_=bexp_t[:, :])
                    for qb in range(QB):
                        nc.sync.dma_start(out=trit_sb[qb], in_=trit_t[qb])
                    for h in range(HPC):
                        nc.sync.dma_start(out=alsp_sb[h], in_=alsp_t[h])
                for f in range(2 * HPC):
                    ps = psqk.tile([P, SCW], F32, tag="psqk")
                    for j in range(KT):
                        nc.tensor.matmul(
                            ps, wqk_sb[f][:, j * P:(j + 1) * P],
                            hp[:, j * SCW:(j + 1) * SCW],
                            start=(j == 0), stop=(j == KT - 1))
                    nc.scalar.activation(
                        out=qk_sb[f][:, c * SCW:(c + 1) * SCW], in_=ps,
                        func=Ident, bias=bqk_sb[:, f:f + 1], scale=1.0)
                for st2 in range(SCW // P):
                    st = c * (SCW // P) + st2
                    ps = psv.tile([P, DPC], F32, tag="psv")
                    for j in range(KT):
                        nc.tensor.matmul(
                            ps, hp[:, j * SCW + st2 * P: j * SCW + (st2 + 1) * P],
                            wv_sb[:, j * DPC:(j + 1) * DPC],
                            start=(j == 0), stop=(j == KT - 1))
                    nc.vector.tensor_add(out=v_sb[st], in0=ps, in1=bv_bc)

        # ---------------- phases 2+3: attention + dense ----------------
        with ExitStack() as ph2:
            pr_pool = ph2.enter_context(tc.tile_pool(name="prow", bufs=9))
            pq_pool = ph2.enter_context(tc.tile_pool(name="pquad", bufs=9))
            sm_pool = ph2.enter_context(tc.tile_pool(name="small", bufs=8))
            wd_pool = ph2.enter_context(tc.tile_pool(name="wd", bufs=1))
            st_pool = ph2.enter_context(tc.tile_pool(name="ostage", bufs=2))
            ps_sc = ph2.enter_context(
                tc.tile_pool(name="ps_sc", bufs=3, space="PSUM"))
            ps_cx = ph2.enter_context(
                tc.tile_pool(name="ps_cx", bufs=1, space="PSUM"))
            ps_d = ph2.enter_context(
                tc.tile_pool(name="ps_d", bufs=2, space="PSUM"))
            if not USE_DMA_T:
                ps_st = ph2.enter_context(
                    tc.tile_pool(name="ps_st", bufs=2, space="PSUM"))

            trit_pool = ph2.enter_context(tc.tile_pool(name="trit", bufs=1))
            trit_sb = [trit_pool.tile([P, P], BF16, tag=f"trit_{qb}",
                                      name=f"trit_{qb}") for qb in range(QB)]
            for qb in range(QB):
                nc.sync.dma_start(out=trit_sb[qb], in_=trit_t[qb])
            wdp_sb = wd_pool.tile([P, HPC * H], BF16, tag="wdp")
            nc.sync.dma_start(out=wdp_sb, in_=wdp_t[:, :])

            prow_t = {}   # (h, qb) -> prob row tile
            quads = {}    # (h, qg) -> list of quad tiles [P, 4, 512]

            def chain(h, qb):
                """scores (+alibi, -b_q, mask) in PSUM -> exp -> normalized
                bf16 prob row."""
                kN = kNq[qb]
                lo0 = (kLoT[h][qb] * P) // 512 * 512
                nt = (kN - lo0 + 511) // 512
                prow = pr_pool.tile([P, S], BF16, tag="prow",
                                    name=f"prow_{h}_{qb}")
                prow_t[(h, qb)] = prow
                strip = sm_pool.tile([P, 4], F32, tag="strip")
                rinv = sm_pool.tile([P, 1], F32, tag="rinv")
                nb = bexp_sb[:, h * QB + qb: h * QB + qb + 1]
                qst = qk_sb[h][:, qb * P:(qb + 1) * P]
                for ti in range(nt):
                    lo = lo0 + 512 * ti
                    N = min(512, kN - lo)
                    ps = ps_sc.tile([P, 512], F32, tag="ps_sc")
                    sl = ps[:, :N]
                    diag = (lo + N == kN)
                    nc.tensor.matmul(
                        sl, qst, qk_sb[HPC + h][:, lo:lo + N],
                        start=True, stop=False)
                    nc.tensor.matmul(
                        sl, ones3[h],
                        alsp_sb[h][:, lo:lo + N],
                        start=False, stop=not diag)
                    if diag:
                        nc.tensor.matmul(
                            ps[:, N - P:N],
                            trit_sb[qb], ident_sb,
                            start=False, stop=True)
                    nc.scalar.activation(
                        out=prow[:, lo:lo + N], in_=sl,
                        func=Exp, bias=nb, scale=1.0,
                        accum_out=strip[:, ti:ti + 1])
                if nt > 1:
                    tot = sm_pool.tile([P, 1], F32, tag="tot")
                    nc.vector.tensor_reduce(
                        out=tot, in_=strip[:, :nt], op=ADD,
                        axis=mybir.AxisListType.X)
                else:
                    tot = strip[:, 0:1]
                nc.vector.reciprocal(out=rinv, in_=tot)
                nc.vector.tensor_scalar_mul(
                    out=prow[:, lo0:kN], in0=prow[:, lo0:kN], scalar1=rinv)

            def transposes(h, qb):
                """prow(h, qb) -> key-major quad slices."""
                kN = kNq[qb]
                qg, qbl = qb // 4, qb % 4
                if qbl == 0:
                    ntile_g = kNq[4 * qg + 3] // P
                    a0 = kLoT[h][4 * qg] // 4
                    quads[(h, qg)] = {
                        a: pq_pool.tile([P, 4, 512], BF16, tag="pquad",
                                        name=f"pq_{h}_{qg}_{a}")
                        for a in range(a0, (ntile_g + 3) // 4)}
                prow = prow_t.pop((h, qb))
                qlist = quads[(h, qg)]
                ntile = kN // P
                t0 = kLoT[h][qb]
                if USE_DMA_T:
                    for t in range(t0, ntile):
                        nc.sync.dma_start(
                            out=qlist[t // 4][:, t % 4, qbl * P:(qbl + 1) * P],
                            in_=prow[:, t * P:(t + 1) * P], transpose=True)
                else:
                    t = t0
                    while t < ntile:
                        t = (t // 4) * 4          # align to quad boundary
                        lo_t = max(t, t0)
                        cnt = min(8, ntile - t)
                        stg = ps_st.tile([P, 8, P], BF16, tag="stg")
                        for i in range(lo_t - t, cnt):
                            nc.tensor.transpose(
                                stg[:, i, :], prow[:, (t + i) * P:(t + i + 1) * P],
                                ident_sb)
                        for half in range((cnt + 3) // 4):
                            i0 = max(4 * half, lo_t - t)
                            i1 = min(4 * half + 4, cnt)
                            if i0 >= i1:
                                continue
                            nc.vector.tensor_copy(
                                out=qlist[t // 4 + half][:, i0 - 4 * half:
                                                         i1 - 4 * half,
                                                         qbl * P:(qbl + 1) * P],
                                in_=stg[:, i0:i1, :])
                        t += cnt

            def pv(h, qg):
                kns = [kNq[4 * qg + i] for i in range(4)]
                t0s = [kLoT[h][4 * qg + i] for i in range(4)]
                ntile = kns[3] // P
                tiles_per = [k // P for k in kns]
                qlist = quads.pop((h, qg))
                cps = ps_cx.tile([P, 512], F32, tag="ps_cx")
                nc.vector.memset(cps, 0.0)
                for t in range(t0s[0], ntile):
                    cov = [i for i in range(4)
                           if tiles_per[i] > t and t0s[i] <= t]
                    if not cov:
                        continue
                    ilo, ihi = cov[0], cov[-1]
                    nc.tensor.matmul(
                        cps[:, ilo * P:(ihi + 1) * P],
                        v_sb[t][:, h * P:(h + 1) * P],
                        qlist[t // 4][:, t % 4, ilo * P:(ihi + 1) * P],
                        start=False, stop=(t == ntile - 1),
                        skip_group_check=True)
                nc.vector.tensor_copy(
                    out=ctx_sb[h][:, qg * 512:(qg + 1) * 512], in_=cps)

            def dense_qb(qb, split_dma=False):
                    stage = st_pool.tile([P, H], BF16, tag="ostage")
                    if split_dma:
                        pass
                    for oc in range(8):
                        ps = ps_d.tile([P, 512], F32, tag="ps_d")
                        for hh in range(HPC):
                            nc.tensor.matmul(
                                ps, ctx_sb[hh][:, qb * P:(qb + 1) * P],
                                wdp_sb[:, hh * H + oc * 512:
                                       hh * H + (oc + 1) * 512],
                                start=(hh == 0), stop=(hh == HPC - 1))
                        if oc % 2 == 0:
                            nc.vector.tensor_copy(
                                out=stage[:, oc * 512:(oc + 1) * 512], in_=ps)
                        else:
                            nc.scalar.copy(
                                out=stage[:, oc * 512:(oc + 1) * 512], in_=ps)
                        if split_dma and oc == 3:
                            nc.sync.dma_start(
                                out=out_t[qb * P:(qb + 1) * P, :H // 2],
                                in_=stage[:, :H // 2])
                    if split_dma:
                        nc.sync.dma_start(
                            out=out_t[qb * P:(qb + 1) * P, H // 2:],
                            in_=stage[:, H // 2:])
                    else:
                        nc.sync.dma_start(
                            out=out_t[qb * P:(qb + 1) * P, :], in_=stage)

            # software-pipelined emission over 16 (qg, h) units, head-major
            # within each query group. Unit-level lag: unit u's four chains
            # are interleaved with unit u-1's transposes; u-1's PV closes at
            # qbl 3. Dense query blocks are spread one-or-two per unit as
            # their query group's ctx completes.
            units = [(qg, h) for qg in range(4) for h in range(HPC)]
            dq = []        # dense qbs ready to emit
            for u, (qg, h) in enumerate(units):
                prev = units[u - 1] if u > 0 else None
                for qbl in range(4):
                    chain(h, 4 * qg + qbl)
                    if prev:
                        transposes(prev[1], 4 * prev[0] + qbl)
                    if qbl == 3 and prev:
                        pv(prev[1], prev[0])
                        if prev[1] == HPC - 1:
                            dq.extend(4 * prev[0] + i for i in range(4))
                for _ in range(2 if len(dq) > 4 else 1):
                    if dq:
                        dense_qb(dq.pop(0))
            prev = units[-1]
            for qbl in range(4):
                transposes(prev[1], 4 * prev[0] + qbl)
            pv(prev[1], prev[0])
            dq.extend(4 * prev[0] + i for i in range(4))
            for i, qb in enumerate(dq):
                dense_qb(qb, split_dma=(i == len(dq) - 1))

    nc.compile()
    return nc


def _host_prep(hidden_states, alibi, attention_mask, w_qkv, b_qkv, w_dense):
    """Returns (kNq, in_maps) for the 8 cores."""
    hidden = np.asarray(hidden_states, np.float32).reshape(S, H)
    mask = np.asarray(attention_mask).reshape(S, S)
    alibi = np.asarray(alibi, np.float32).reshape(NH, S)
    w_qkv = np.asarray(w_qkv, np.float32)
    b_qkv = np.asarray(b_qkv, np.float32)
    w_dense = np.asarray(w_dense, np.float32)

    allowed = ~mask
    assert allowed.any(axis=1).all(), "fully-masked row"
    limit = S - np.argmax(allowed[:, ::-1], axis=1)      # last allowed + 1
    recon = np.arange(S)[None, :] >= limit[:, None]
    if not np.array_equal(mask, recon):
        raise NotImplementedError("mask is not suffix-structured")
    kNq = []
    for qb in range(QB):
        lb = limit[qb * P:(qb + 1) * P]
        kN = int(math.ceil(lb.max() / P) * P)
        if lb.min() < kN - P:
            raise NotImplementedError("mask boundary spans >128 cols in block")
        kNq.append(kN)
    if any(kNq[i] > kNq[i + 1] for i in range(QB - 1)):
        raise NotImplementedError("non-monotone key ranges")

    bf = ml_dtypes.bfloat16
    hpk = np.ascontiguousarray(
        hidden.reshape(NCH, SCW, KT, P).transpose(0, 3, 2, 1)
    ).reshape(NCH, P, KT * SCW).astype(bf)
    ident = np.eye(P, dtype=np.float32).astype(bf)
    col = np.arange(S)

    # causal diagonal mask tiles, transposed for use as matmul stationary:
    # trit[qb][k, q] = NEG where key kN-P+k is masked for query q
    trit = np.zeros((QB, P, P), np.float32)
    for qb in range(QB):
        kN = kNq[qb]
        lb = limit[qb * P:(qb + 1) * P]
        cc = col[kN - P:kN]
        trit[qb] = np.where(cc[:, None] >= lb[None, :], NEG, 0.0)
    trit = trit.astype(bf)

    wr = w_qkv.reshape(NH, 3, HD, H)
    br = b_qkv.reshape(NH, 3, HD)

    in_maps = []
    all_kLoT = None
    for c in range(NCORES):
        heads = [c + NCORES * j for j in range(HPC)]
        hs = np.asarray(heads)
        Wq = wr[hs, 0].reshape(DPC, H) * INV_NORM
        Wk = wr[hs, 1].reshape(DPC, H)
        Wv = wr[hs, 2].reshape(DPC, H)
        WQK = np.concatenate([Wq, Wk], axis=0)           # [1024, H]
        wqk_c = np.ascontiguousarray(
            WQK.reshape(2 * HPC, P, KT, P).transpose(0, 3, 2, 1)
        ).reshape(2 * HPC, P, KT * P).astype(bf)
        wv_c = np.ascontiguousarray(
            Wv.reshape(DPC, KT, P).transpose(2, 1, 0)
        ).reshape(P, KT * DPC).astype(bf)
        bq = br[hs, 0].reshape(-1) * INV_NORM
        bk = br[hs, 1].reshape(-1)
        bqk_c = np.ascontiguousarray(
            np.concatenate([bq, bk]).reshape(2 * HPC, P).T)
        bv_c = br[hs, 2].reshape(1, DPC)

        al_c = alibi[hs].astype(np.float32)               # [HPC, S]
        # exact 3-way bf16 split of alibi
        a_hi = al_c.astype(bf).astype(np.float32)
        r1 = al_c - a_hi
        a_mid = r1.astype(bf).astype(np.float32)
        a_lo = (r1 - a_mid).astype(bf)
        alsp_c = np.stack(
            [a_hi.astype(bf), a_mid.astype(bf), a_lo], axis=1)  # [HPC,3,S]

        cmax = np.maximum.accumulate(al_c, axis=1)
        bexp_c = np.zeros((P, HPC * QB), np.float32)
        kLoT_c = []
        for h in range(HPC):
            b_row = cmax[h, limit - 1] + CPAD
            klo_h = []
            for qb in range(QB):
                bexp_c[:, h * QB + qb] = -b_row[qb * P:(qb + 1) * P]
                # keys whose softmax weight is < ~e^-19 for every query in
                # the block (qk slack 25 + prob floor e^-23): contribute
                # < 1e-5 total mass, far below the kernel's 5e-3 error
                bmin = b_row[qb * P:(qb + 1) * P].min()
                live = al_c[h] >= (bmin - 48.0)
                k0 = int(np.argmax(live)) if live.any() else 0
                klo_h.append(min(k0 // P, kNq[qb] // P - 1))
            kLoT_c.append(tuple(klo_h))
        kLoT_c = tuple(kLoT_c)
        if all_kLoT is None:
            all_kLoT = kLoT_c
        else:
            # one SPMD program for all cores: take the elementwise min
            all_kLoT = tuple(
                tuple(min(a, b) for a, b in zip(ra, rb))
                for ra, rb in zip(all_kLoT, kLoT_c))
        dcols = np.concatenate(
            [np.arange(g * HD, (g + 1) * HD) for g in heads])
        wdp_c = np.ascontiguousarray(
            w_dense[:, dcols].reshape(H, HPC, P)
            .transpose(2, 1, 0)).reshape(P, HPC * H).astype(bf)
        in_maps.append({
            "hpk": hpk, "wqk": wqk_c, "wv": wv_c, "bqk": bqk_c, "bv": bv_c,
            "alsp": alsp_c, "trit": trit, "bexp": bexp_c, "ident": ident,
            "wdp": wdp_c,
        })
    return (tuple(kNq), all_kLoT), in_maps


def kernel(hidden_states, residual, alibi, attention_mask,
           w_qkv, b_qkv, w_dense, b_dense):
    key, in_maps = _host_prep(hidden_states, alibi, attention_mask,
                              w_qkv, b_qkv, w_dense)
    if key not in _CACHE:
        _CACHE[key] = _build(key)
    nc = _CACHE[key]
    res = run_bass_kernel_spmd(nc, in_maps, list(range(NCORES)))
    acc = res.results[0]["out_part"].astype(np.float32)
    for c in range(1, NCORES):
        acc += res.results[c]["out_part"].astype(np.float32)
    out = acc + np.asarray(b_dense, np.float32)[None, :]
    out = out + np.asarray(residual, np.float32).reshape(S, H)
    return out.reshape(B, S, H).astype(np.float32)
